# revision 45
# baseline (speedup 1.0000x reference)
"""Trainium2 Bass kernel for nn_Aggregation (SAN-style position-dependent
3x3 depthwise aggregation with share_planes=8).

  out[n, c, h, w] = sum_k input[n, c, h+dh(k), w+dw(k)] * weight[n, c//8, k, h*W+w]

Sharding: data-parallel over batch N=8 across the 8 NeuronCores (one image
per core, no collectives).

Per-core design (input [256,56,56], weight [32,9,3136] per image):
  - SBUF partition p = q*32 + g: q in 0..3 = 14-row quarter of the image,
    g in 0..31 = weight group. The 8 share-channels of a group live in the
    free dimension, so each weight element is read via a stride-0 broadcast
    AP instead of being replicated.
  - The host pre-packs ONE flat fp16 slab per partition:
      [ x chunk0 (s=0..3, 4x900) | weight (9x784) | identity row (128) |
        x chunk1 (s=4..7, 4x900) ]
    where each 900-elem x block is a zero-padded flat image quarter
    (guard + 16 rows [14 + 2 halo] * 56 cols + guard + pad), so each tap
    (dh, dw) is a single contiguous 784-slice at offset 1 + (dh+1)*56 + dw.
    Column wrap-around reads are neutralized by zeroing the weight's edge
    columns host-side (those taps multiply out-of-image zero padding in the
    exact computation).  Packing gives the DMA maximal contiguous runs per
    partition.  Each DMA-issuing engine owns one serial queue stream with a
    multi-us fixed startup, and consumers wait on whole-DMA completion, so
    the input is split by consumption stage across three queues: x chunk 0
    (the compute gate) on sync-HWDGE, identity + weight planes in three
    consumption-ordered groups on scalar-HWDGE, x chunk 1 on the
    late-starting gpsimd-SWDGE queue.
  - fp16 storage: DVE tensor_tensor runs in 2x perf mode (needs 16-bit,
    step 1, 4B-aligned APs -> a second, one-element-shifted copy of the x
    slab, built on-chip by ScalarE, gives every tap an even base offset).
    ALU math is fp32 internally.
  - Compute: VectorE does only the 9 tap multiplies per chunk (2x mode);
    TensorE accumulates the 9 product arrays into PSUM with identity-
    stationary matmuls (fp32 accumulation, start/stop has_written groups),
    concurrently on its own SBUF ports; ScalarE drains PSUM -> SBUF with an
    fp16 downcast.  Work is split into 2 share-axis chunks (PSUM capacity =
    8 banks, 7 x 448-col tiles per chunk), overlapping chunk-1's DMA with
    chunk-0 compute.  At the tail the idle VectorE takes part of the last
    drain and the output DMA is split so it streams during the drain.
"""

import numpy as np

N, C, H, W = 8, 256, 56, 56
G, KK, L = 32, 9, 3136          # weight groups, taps, spatial
SHARE = 8                        # C // G
Q = 4                            # row-quarters
RQ = H // Q                      # 14 rows per quarter
LQ = RQ * W                      # 784 pixels per quarter
XA = 900                         # guard + 16*56 + guard + pad (even)

DTYPE = "float16"                # on-chip storage dtype
SPLIT = 2                        # share-axis chunks (overlap DMA/compute)
SC = SHARE // SPLIT              # share-channels per chunk
# tap order: xa-based taps (dw=+-1) first so compute can start before the
# on-chip xb shift-copies finish; xb-based taps (dw=0) last.
TAP_ORDER = [0, 2, 3, 5, 6, 8, 1, 4, 7]
MM = 448                         # matmul free-dim tile (7 * 448 = 3136)

# packed input slab column offsets (fp16 elements per partition).
# weight planes are stored in TAP_ORDER so they stream off HBM in exactly
# the order compute consumes them (region-based deps let tap j start as
# soon as its plane lands).
OFF_X0 = 0
OFF_ID = OFF_X0 + SC * XA        # 3600
OFF_WT = OFF_ID + 128            # 3728
OFF_X1 = OFF_WT + KK * LQ        # 10784
SLAB = OFF_X1 + SC * XA          # 14384

_CACHE = {}


def _build():
    import concourse.bacc as bacc
    import concourse.mybir as mybir
    import concourse.tile as tile

    dt = getattr(mybir.dt, DTYPE)

    nc = bacc.Bacc("TRN2", target_bir_lowering=False, debug=False)
    inp = nc.dram_tensor("inp", [128, SLAB], dt, kind="ExternalInput")
    out = nc.dram_tensor("out", [128, SHARE, LQ], dt, kind="ExternalOutput")

    with tile.TileContext(nc) as tc:
        with (
            tc.tile_pool(name="main", bufs=1) as pool,
            tc.tile_pool(name="prod", bufs=4) as ppool,
            tc.tile_pool(name="psum", bufs=1, space="PSUM") as psum_pool,
        ):
            inbuf = pool.tile([128, SLAB], dt)
            xb = pool.tile([128, SHARE, XA - 4], dt)

            # Each DMA-issuing engine owns ONE hardware queue stream
            # (sync-HWDGE, scalar-HWDGE, gpsimd-SWDGE) and per-queue
            # throughput caps well below HBM bandwidth — so stripe the
            # chunk-0 gate (x0 + weights + identity) across all three
            # queues, then chunk-1's x behind it on the same queues
            # (engine program order keeps the gate data first).
            # Queue plan (each issuing engine = one serial queue stream,
            # ~140GB/s, ~5-7us fixed startup after arming; a consumer waits
            # on whole-DMA completion, so split by consumption stage):
            #   sync-HWDGE:   x0 -> chunk-0 compute gate; queue then free
            #                 for the output DMAs.
            #   scalar-HWDGE: ident + weight planes in 3 consumption-order
            #                 groups, so early taps unblock first.
            #   gpsimd-SWDGE: x1 (this queue starts ~12us late, but chunk-1
            #                 isn't needed until ~27us).
            # 4-element queue warm-ups: the HWDGE queues' first-use startup
            # (~5us) otherwise serializes across queues. Dest is the xb
            # tile, which is fully overwritten by the ScalarE shift-copies
            # below before anything reads it — so the real input DMAs have
            # no region overlap with the warm-ups and arm without waiting.
            nc.sync.dma_start(out=xb[:, SC, 0:4], in_=inp.ap()[:, 0:4])
            nc.scalar.dma_start(out=xb[:, 0, 0:4], in_=inp.ap()[:, 0:4])
            # Per-queue streaming is only ~140GB/s, so the compute gate
            # (x0 + ident + wt plane 0) is striped across all three queues;
            # later weight planes ladder in just ahead of their taps.
            # Queue first-data order is gpsimd ~9.2us < sync ~10.9 < scalar
            # ~12.2 (SWDGE desc-gen starts immediately; HWDGE inits
            # serialize even with warm-ups) — so the binding gate piece
            # (ident + first two weight planes) rides sync, x0 halves ride
            # scalar+gpsimd, later planes ladder behind on scalar.
            for eng, a, b in (
                (nc.scalar, OFF_X0, OFF_X0 + 1800),          # x0 s0:2
                (nc.gpsimd, OFF_X0 + 1800, OFF_ID),          # x0 s2:4
                (nc.sync, OFF_ID, OFF_WT + 2 * LQ),          # ident + wt taps 0-1
                (nc.scalar, OFF_WT + 2 * LQ, OFF_WT + 4 * LQ),
                (nc.scalar, OFF_WT + 4 * LQ, OFF_WT + 6 * LQ),
                (nc.scalar, OFF_WT + 6 * LQ, OFF_X1),
                (nc.gpsimd, OFF_X1, SLAB),                   # x1
            ):
                eng.dma_start(out=inbuf[:, a:b], in_=inp.ap()[:, a:b])

            xa_views = [
                inbuf[:, OFF_X0 : OFF_X0 + SC * XA].rearrange(
                    "p (s l) -> p s l", s=SC
                ),
                inbuf[:, OFF_X1 : OFF_X1 + SC * XA].rearrange(
                    "p (s l) -> p s l", s=SC
                ),
            ]
            wt = inbuf[:, OFF_WT : OFF_WT + KK * LQ].rearrange(
                "p (k l) -> p k l", k=KK
            )
            ident = inbuf[:, OFF_ID : OFF_ID + 128]

            # xb = x shifted by one element: gives dw=0 taps an even base;
            # built on-chip by the otherwise-idle ScalarE.
            for c in range(SPLIT):
                nc.scalar.copy(
                    xb[:, c * SC : (c + 1) * SC, :], xa_views[c][:, :, 1 : XA - 3]
                )

            def x_ap_for(c, k):
                dh, dw = k // 3 - 1, k % 3 - 1
                if dw == 0:
                    base = (dh + 1) * W      # even; xb = xa shifted by 1
                    return xb[:, c * SC : (c + 1) * SC, base : base + LQ]
                base = 1 + (dh + 1) * W + dw  # even by construction
                return xa_views[c][:, :, base : base + LQ]

            outbuf = pool.tile([128, SHARE, LQ], dt)
            nhalf = (SC * LQ) // MM          # matmul tiles per chunk
            for c in range(SPLIT):
                s0, s1 = c * SC, (c + 1) * SC
                banks = [
                    psum_pool.tile(
                        [128, MM], mybir.dt.float32,
                        name=f"bank{c}_{t}", tag=f"bank{t}",
                    )
                    for t in range(nhalf)
                ]
                for j, k in enumerate(TAP_ORDER):
                    prod = ppool.tile([128, SC * LQ], dt)
                    prod_s = prod[:].rearrange("p (s l) -> p s l", s=SC)
                    # weight planes are stored in TAP_ORDER -> plane j
                    w_ap = wt[:, j : j + 1, :].broadcast_to([128, SC, LQ])
                    nc.vector.tensor_mul(prod_s, x_ap_for(c, k), w_ap)
                    for t in range(nhalf):
                        nc.tensor.matmul(
                            banks[t][:],
                            ident,
                            prod[:, t * MM : (t + 1) * MM],
                            start=(j == 0),
                            stop=(j == KK - 1),
                            skip_group_check=True,
                        )
                ob = outbuf[:, s0:s1, :].rearrange("p s l -> p (s l)")
                if c == SPLIT - 1:
                    # tail: VectorE is idle after its last mult — split the
                    # PSUM drain between VectorE and ScalarE, and stream the
                    # first part of the output while the rest drains.
                    ofl = out.ap().rearrange("p s l -> p (s l)")
                    col0 = s0 * LQ
                    for t in range(4):
                        nc.vector.tensor_copy(
                            out=ob[:, t * MM : (t + 1) * MM], in_=banks[t][:]
                        )
                    for t in range(4, nhalf):
                        nc.scalar.copy(ob[:, t * MM : (t + 1) * MM], banks[t][:])
                    # the two tail pieces stream on different queues in
                    # parallel (both idle by now)
                    nc.sync.dma_start(
                        out=ofl[:, col0 : col0 + 4 * MM], in_=ob[:, 0 : 4 * MM]
                    )
                    nc.scalar.dma_start(
                        out=ofl[:, col0 + 4 * MM : col0 + nhalf * MM],
                        in_=ob[:, 4 * MM : nhalf * MM],
                    )
                else:
                    for t in range(nhalf):
                        nc.scalar.copy(ob[:, t * MM : (t + 1) * MM], banks[t][:])
                    nc.sync.dma_start(
                        out=out.ap()[:, s0:s1, :], in_=outbuf[:, s0:s1, :]
                    )

    nc.compile()
    return nc


def _get_nc():
    if "nc" not in _CACHE:
        _CACHE["nc"] = _build()
    return _CACHE["nc"]


def _prep_shards(input, weight):
    np_dt = np.dtype(DTYPE)
    # padded image per (g, s): rows -1..56 zero-padded
    inp = np.asarray(input).reshape(N, G, SHARE, H, W)
    pad = np.zeros((N, G, SHARE, H + 2, W), dtype=np_dt)
    pad[:, :, :, 1 : H + 1, :] = inp
    # x slab: [N, q, g, s, XA]
    xh = np.zeros((N, Q, G, SHARE, XA), dtype=np_dt)
    for q in range(Q):
        xh[:, q, :, :, 1 : 1 + 16 * W] = pad[:, :, :, q * RQ : q * RQ + 16, :].reshape(
            N, G, SHARE, 16 * W
        )
    xh = xh.reshape(N, 128, SHARE, XA)

    # weight: [N, (q g), k, LQ] with out-of-image edge columns zeroed
    wh = np.asarray(weight).astype(np_dt).reshape(N, G, KK, H, W)
    for k in range(KK):
        dwk = k % 3 - 1
        if dwk == -1:
            wh[:, :, k, :, 0] = 0
        elif dwk == 1:
            wh[:, :, k, :, W - 1] = 0
    wh = (
        wh.reshape(N, G, KK, Q, LQ)
        .transpose(0, 3, 1, 2, 4)
        .reshape(N, 128, KK * LQ)
    )

    wh = wh.reshape(N, 128, KK, LQ)[:, :, TAP_ORDER, :]   # consumption order

    slab = np.empty((N, 128, SLAB), dtype=np_dt)
    slab[:, :, OFF_X0:OFF_ID] = xh[:, :, :SC, :].reshape(N, 128, SC * XA)
    slab[:, :, OFF_ID:OFF_WT] = np.eye(128, dtype=np_dt)[None]
    slab[:, :, OFF_WT:OFF_X1] = wh.reshape(N, 128, KK * LQ)
    slab[:, :, OFF_X1:SLAB] = xh[:, :, SC:, :].reshape(N, 128, SC * XA)
    return [{"inp": np.ascontiguousarray(slab[n])} for n in range(N)]


def _unpack_out(res_list):
    # res: [128, SHARE, LQ] per core -> (N, C, H, W) float32
    o = np.stack([r["out"] for r in res_list], axis=0).astype(np.float32)
    o = o.reshape(N, Q, G, SHARE, LQ).transpose(0, 2, 3, 1, 4)
    return np.ascontiguousarray(o.reshape(N, C, H, W))


def kernel(input, weight):
    from concourse.bass_utils import run_bass_kernel_spmd

    nc = _get_nc()
    in_maps = _prep_shards(input, weight)
    res = run_bass_kernel_spmd(nc, in_maps, core_ids=list(range(N)))
    return _unpack_out(res.results)
